# revision 2
# baseline (speedup 1.0000x reference)
import numpy as np
import ml_dtypes
import concourse.bass as bass
import concourse.tile as tile
from concourse import bacc, mybir
from concourse.bass_utils import run_bass_kernel_spmd

# Block self-attention: 32x32 areas of 4x4 blocks of 8x8 pixels.
# Sharding: 8 cores = 4 batches x 2 H-halves of 256 rows.
#
# v1 architecture (vs baseline): no DRAM bounce.
#  - host packs x as [128, units*4096] bf16: rows (h, ch) h = pixel-half
#    (p = h*32 + p'), cols (u, p', a, i).
#  - conv1 2-way packed, block-diag stationary [128, 96]; output rows
#    grouped by channel type: q 0:32, t 32:64, g 64:96, each (h, c16).
#  - gather y -> z_q/z_t/z_g [(a i), (p' h c)] via THREE xbar
#    DMA-transposes (SBUF->SBUF, no HBM traffic); 32-aligned bases.
#  - 16 PE transposes of contiguous z slices -> qkb [(p'4 h c), (a i)];
#    scores = 8 accumulating matmuls + rank-9 mask matmul; exp/rowsum;
#    o = eT @ z_g halves.
#  - 16 small PE transposes of o -> oT; conv2 packed with block-diag
#    stationaries covering (pm, h) pixel phases -> osum [128, 4096]
#  - out [128, units*4096] bf16; host unpermutes, adds residual + b_out.

F32 = mybir.dt.float32
BF16 = mybir.dt.bfloat16
AF = mybir.ActivationFunctionType
BF = ml_dtypes.bfloat16

_cached = {}


def _build_nc(units=16):
    nc = bacc.Bacc("TRN2", target_bir_lowering=False, debug=False, num_devices=8)
    xs = nc.dram_tensor("xs", [128, units * 4096], BF16, kind="ExternalInput").ap()
    w1s = nc.dram_tensor("w1s", [128, 96], BF16, kind="ExternalInput").ap()
    b1s = nc.dram_tensor("b1s", [96, 1], F32, kind="ExternalInput").ap()
    w2s = [
        nc.dram_tensor(f"w2s{i}", [128, 128], BF16, kind="ExternalInput").ap()
        for i in range(4)
    ]
    identb = nc.dram_tensor("identb", [128, 128], BF16, kind="ExternalInput").ap()
    mbu = nc.dram_tensor("mbu", [9, 128], BF16, kind="ExternalInput").ap()
    mbv = nc.dram_tensor("mbv", [9, 128], BF16, kind="ExternalInput").ap()
    out = nc.dram_tensor("out", [128, units * 4096], BF16, kind="ExternalOutput").ap()

    with tile.TileContext(nc) as tc:
        with (
            tc.tile_pool(name="const", bufs=1) as cpool,
            tc.tile_pool(name="xio", bufs=2) as xio,
            tc.tile_pool(name="stage", bufs=2) as stg,
            tc.tile_pool(name="sm", bufs=2) as sm,
            tc.tile_pool(name="pcv", bufs=3, space="PSUM") as pcv,
            tc.tile_pool(name="ptp", bufs=2, space="PSUM") as ptp,
            tc.tile_pool(name="patt", bufs=2, space="PSUM") as patt,
        ):
            w1s_t = cpool.tile([128, 96], BF16)
            nc.sync.dma_start(w1s_t, w1s)
            b1s_t = cpool.tile([96, 1], F32)
            nc.sync.dma_start(b1s_t, b1s)
            w2s0_t = cpool.tile([128, 128], BF16)
            nc.sync.dma_start(w2s0_t, w2s[0])
            w2s1_t = cpool.tile([128, 128], BF16)
            nc.sync.dma_start(w2s1_t, w2s[1])
            w2s2_t = cpool.tile([128, 128], BF16)
            nc.sync.dma_start(w2s2_t, w2s[2])
            w2s3_t = cpool.tile([128, 128], BF16)
            nc.sync.dma_start(w2s3_t, w2s[3])
            w2_t = [w2s0_t, w2s1_t, w2s2_t, w2s3_t]
            id_t = cpool.tile([128, 128], BF16)
            nc.sync.dma_start(id_t, identb)
            mbu_t = cpool.tile([9, 128], BF16)
            nc.sync.dma_start(mbu_t, mbu)
            mbv_t = cpool.tile([9, 128], BF16)
            nc.sync.dma_start(mbv_t, mbv)

            state = {}

            def stage_in(u):
                x_t = xio.tile([128, 4096], BF16, tag="x", bufs=4)
                nc.gpsimd.dma_start(x_t, xs[:, 4096 * u : 4096 * (u + 1)])
                state[("x", u)] = x_t

            def stage_a(u):
                # conv1, 2-way row-packed via block-diagonal stationary
                x_t = state.pop(("x", u))
                y_t = stg.tile([96, 4096], BF16, tag="y", bufs=3)
                for t in range(8):
                    cps = pcv.tile([128, 512], F32, tag="cv")
                    nc.tensor.matmul(
                        cps[0:96, :],
                        w1s_t,
                        x_t[:, 512 * t : 512 * t + 512],
                        start=True,
                        stop=True,
                    )
                    ysl = y_t[:, 512 * t : 512 * t + 512]
                    if t % 2 == 0:
                        nc.scalar.activation(ysl, cps[0:96, :], AF.Identity, bias=b1s_t)
                    else:
                        nc.vector.tensor_scalar_add(ysl, cps[0:96, :], b1s_t)
                # gather y [(ct h c), (p' a i)] -> z [(a i), (ct p' h c)]
                z_t = stg.tile([128, 3072], BF16, tag="z", bufs=3)
                for ct in range(3):
                    zsl = z_t[:, 1024 * ct : 1024 * (ct + 1)].rearrange(
                        "p (m w) -> p m w", m=32, w=32
                    )
                    nc.sync.dma_start(
                        zsl, y_t[32 * ct : 32 * ct + 32, :], transpose=True
                    )
                state[("z", u)] = z_t

            def stage_b(u):
                z_t = state[("z", u)]
                # transpose q/t chunks -> qkb [(p'4 h c), (a i)]
                qkb = stg.tile([128, 2048], BF16, tag="qkb", bufs=3)
                for grp in range(4):
                    tp = ptp.tile([128, 512], BF16, tag="tp")
                    for m in range(4):
                        k = grp * 4 + m  # 0..7 q chunks, 8..15 t chunks
                        ct, kk = divmod(k, 8)
                        src = z_t[:, 1024 * ct + 128 * kk : 1024 * ct + 128 * kk + 128]
                        nc.tensor.transpose(tp[:, 128 * m : 128 * m + 128], src, id_t)
                    dst = qkb[:, 512 * grp : 512 * grp + 512]
                    if grp % 2 == 0:
                        nc.vector.tensor_copy(dst, tp)
                    else:
                        nc.scalar.activation(dst, tp, AF.Copy)

                # scores (8 areas batched; block-diag mask via rank-9 matmul)
                sps = patt.tile([128, 512], F32, tag="att")
                s_ap = sps[:, 0:128]
                for k in range(8):
                    nc.tensor.matmul(
                        s_ap,
                        qkb[:, 128 * k : 128 * k + 128],
                        qkb[:, 1024 + 128 * k : 1024 + 128 * k + 128],
                        start=(k == 0),
                        stop=False,
                    )
                nc.tensor.matmul(s_ap, mbu_t, mbv_t, start=False, stop=True)

                e_t = sm.tile([128, 128], BF16, tag="e")
                nc.scalar.activation(e_t, s_ap, AF.Exp)
                r_t = sm.tile([128, 1], F32, tag="r")
                nc.vector.reduce_sum(r_t, e_t, axis=mybir.AxisListType.X)
                nc.vector.reciprocal(r_t, r_t)
                # fold softmax normalization into e before transposing
                es_t = sm.tile([128, 128], BF16, tag="es")
                nc.vector.tensor_scalar_mul(es_t, e_t, r_t)

                etp = ptp.tile([128, 512], BF16, tag="tp")
                nc.tensor.transpose(etp[:, 0:128], es_t, id_t)
                eT = sm.tile([128, 128], BF16, tag="eT")
                nc.vector.tensor_copy(eT, etp[:, 0:128])

                # oT chunks [(pm h c), (a i)] = z_g_chunk^T @ eT directly
                # (no o transposes needed; conv2-ready orientation)
                oT = stg.tile([128, 1024], BF16, tag="oT", bufs=3)
                for hf in range(2):
                    otp = pcv.tile([128, 512], F32, tag="cv")
                    for kk in range(4):
                        k = hf * 4 + kk
                        nc.tensor.matmul(
                            otp[:, 128 * kk : 128 * kk + 128],
                            z_t[:, 2048 + 128 * k : 2048 + 128 * k + 128],
                            eT,
                            start=True,
                            stop=True,
                        )
                    dst = oT[:, 512 * hf : 512 * hf + 512]
                    if hf == 0:
                        nc.vector.tensor_copy(dst, otp)
                    else:
                        nc.scalar.activation(dst, otp, AF.Copy)
                state.pop(("z", u))
                state[("oT", u)] = oT

            def stage_c(u):
                # phase-packed conv2: contraction over full (pm4, h, c) 128
                oT = state.pop(("oT", u))
                osum = xio.tile([128, 4096], BF16, tag="os", bufs=3)
                n = 0
                for sl in range(2):
                    for X in range(4):
                        c2 = pcv.tile([128, 512], F32, tag="cv")
                        nc.tensor.matmul(
                            c2,
                            w2_t[X],
                            oT[:, 512 * sl : 512 * sl + 512],
                            start=True,
                            stop=True,
                        )
                        osl = osum[:, 512 * n : 512 * n + 512]
                        if n % 2 == 0:
                            nc.vector.tensor_copy(osl, c2)
                        else:
                            nc.scalar.activation(osl, c2, AF.Copy)
                        n += 1
                nc.gpsimd.dma_start(out[:, 4096 * u : 4096 * (u + 1)], osum)

            # software pipeline: prefetch | conv1+gather | attention | conv2
            for s in range(units + 3):
                if s < units:
                    stage_in(s)
                if 1 <= s < units + 1:
                    stage_a(s - 1)
                if 2 <= s < units + 2:
                    stage_b(s - 2)
                if 3 <= s:
                    stage_c(s - 3)
    nc.compile()
    return nc


# oT chunk k (0..7) covers p' in [4k, 4k+4); chunk partitions (pm, h, c).
# conv2 moving slice sl = chunk-half; stationary X selects pm phase;
# out partition = w*64 + oc with w = h.
# => p' = 4*(4*sl + kk) + X = 16*sl + 4*kk + X


def _pack_x(xc):
    # xc [64, 256, 512] f32 -> xs [128, 65536] bf16
    # rows (prh, ch); cols (u=(ru,cu), p'=(prl,pc), a, i=(br,bc))
    t = xc.reshape(64, 8, 4, 2, 4, 2, 8, 4, 8)
    t = t.transpose(3, 0, 1, 5, 4, 8, 6, 2, 7)
    return np.ascontiguousarray(t.reshape(128, 65536).astype(BF))


def _unpack_out(o):
    # o [128, 65536] f32 -> [64, 256, 512]
    # rows (h, oc); cols (ru, cu, sl, X, kk, a, br, bc); p' = 16sl+4kk+X
    t = o.reshape(2, 64, 8, 2, 2, 4, 2, 2, 8, 4, 4)
    # (h0, oc1, ru2, cu3, sl4, X5, kh6, kl7, a8, br9, bc10)
    # row = ru*32 + br*8 + pr;  pr = h*4 + sl*2 + kh
    # col = cu*256 + a*32 + bc*8 + pc;  pc = kl*4 + X
    t = t.transpose(1, 2, 9, 0, 4, 6, 3, 8, 10, 7, 5)
    # (oc, ru, br, h, sl, kh, cu, a, bc, kl, X)
    return t.reshape(64, 256, 512)


def _make_common(w_ptg, b_ptg, w_out):
    # conv1 stationary: rows (h, ch) -> cols (ct, h, c)
    w1s = np.zeros((128, 96), dtype=BF)
    b1s = np.zeros((96, 1), np.float32)
    wT = w_ptg.T.astype(BF)  # [64 ch, 48 c] with c = (ct, c16)
    for ct in range(3):
        for h in range(2):
            w1s[64 * h : 64 * h + 64, 32 * ct + 16 * h : 32 * ct + 16 * h + 16] = (
                wT[:, 16 * ct : 16 * ct + 16]
            )
            b1s[32 * ct + 16 * h : 32 * ct + 16 * h + 16, 0] = b_ptg[
                16 * ct : 16 * ct + 16
            ]
    # conv2 stationaries: rows (pm4, h, c) = 128; stationary X covers
    # pm == X with cols (w, oc), w = h
    w2sl = []
    for X in range(4):
        m = np.zeros((128, 128), dtype=BF)
        for w in range(2):
            m[32 * X + 16 * w : 32 * X + 16 * w + 16, 64 * w : 64 * w + 64] = (
                w_out.T.astype(BF)
            )
        w2sl.append(m)
    mc = np.zeros((8, 128), dtype=BF)
    for a in range(8):
        mc[a, 16 * a : 16 * a + 16] = 100.0
    mbu = np.concatenate([np.full((1, 128), -100.0, dtype=BF), mc])
    mbv = np.concatenate([np.full((1, 128), 100.0, dtype=BF), mc])
    return {
        "w1s": w1s,
        "b1s": b1s,
        "w2s0": w2sl[0],
        "w2s1": w2sl[1],
        "w2s2": w2sl[2],
        "w2s3": w2sl[3],
        "identb": np.eye(128, dtype=BF),
        "mbu": mbu,
        "mbv": mbv,
    }


def kernel(x, w_ptg, b_ptg, w_out, b_out):
    x = np.asarray(x, dtype=np.float32)
    w_ptg = np.asarray(w_ptg, dtype=np.float32)
    b_ptg = np.asarray(b_ptg, dtype=np.float32)
    w_out = np.asarray(w_out, dtype=np.float32)
    b_out = np.asarray(b_out, dtype=np.float32)

    # pad vector: w_ptg @ xpad + b_ptg = 0 so conv1 output is 0 at pad pixels
    xpad, *_ = np.linalg.lstsq(w_ptg, -b_ptg, rcond=None)
    xp = np.empty((4, 64, 512, 512), np.float32)
    xp[:] = xpad.astype(np.float32)[None, :, None, None]
    xp[:, :, :504, :504] = x

    common = _make_common(w_ptg, b_ptg, w_out)
    in_maps = []
    for b in range(4):
        for hh in range(2):
            xc = xp[b, :, 256 * hh : 256 * hh + 256, :]
            in_maps.append({"xs": _pack_x(xc), **common})

    if "nc" not in _cached:
        _cached["nc"] = _build_nc()
    res = run_bass_kernel_spmd(_cached["nc"], in_maps, list(range(8)))
    _cached["last_res"] = res

    outp = np.empty((4, 64, 512, 512), np.float32)
    for idx in range(8):
        b, hh = divmod(idx, 2)
        o = np.asarray(res.results[idx]["out"]).astype(np.float32)
        outp[b, :, 256 * hh : 256 * hh + 256, :] = _unpack_out(o)
    out = outp[:, :, :504, :504] + b_out[None, :, None, None] + x
    return np.ascontiguousarray(out.astype(np.float32))


if __name__ == "__main__":
    import reference

    inputs = {k: np.asarray(v) for k, v in reference.setup_inputs().items()}
    got = kernel(**inputs)
    exp = np.asarray(reference.reference(**inputs))
    err = np.abs(got - exp).max() / np.abs(exp).max()
    print("Relative error:", err)
